# revision 21
# baseline (speedup 1.0000x reference)
"""Bahdanau additive attention on 8 Trainium2 NeuronCores.

reference:
    dec_proj = hidden_dec @ Wa                       # [B, U]
    enc_proj = einsum("bse,eu->bsu", outp_enc, Ua)   # [B, S, U]
    e        = tanh(enc_proj + dec_proj[:, None, :])
    scores   = einsum("bsu,u->bs", e, Va)
    alpha    = softmax(scores, axis=-1)
    context  = einsum("bs,bse->be", alpha, outp_enc)  # [B, E]

Sharding: data-parallel on batch. B=32 over 8 cores -> 4 batches/core.
Weights (Wa, Ua, Va) replicated; no collectives.

Per-core plan (4 local batches, S=1024, E=U=D=512), processed
half-by-half (512 s at a time) across all 4 batches:
  - host ships TWO fp16 layouts of the enc shard: transposed [e,s] for
    the enc_proj matmuls and natural [s,e] for the context matmuls
    (halves HBM traffic vs f32 natural).
  - enc_proj in layout [u, s]: lhsT = Ua chunk, rhs = encT; tanh's
    dec_proj bias is a per-partition scalar on the scalar engine.
  - scores/context use PE column tiling: each local batch b owns
    partition rows [32b, 32b+32) of a shared PSUM tile, so the four
    M=1 matmuls of a wave land in different 32-column groups of the
    PE array and execute concurrently (issue waves batch-major).
  - scores lhsT is Va zero-padded to M=32 so every PSUM row is
    written: junk alpha rows become exp(0)=1 (finite), which keeps
    the alpha transposes NaN-free.
  - one shared alpha tile [128, S] holds all 4 batches (rows 32b);
    softmax is ONE exp per half with accum_out, 8 PE transposes total,
    and a single fused reciprocal-scale evacuation of the context.
  - alpha stays unnormalized; 1/sum is folded into the final scale.
    No max-subtraction: |scores| <= ||Va||_1 with tanh in (-1,1).
fp16 on the scores path adds ~2-4e-4 relative error; fp16 natural
layout adds ~1e-4 on the context sum.
"""

import os

import numpy as np

import concourse.bacc as bacc
import concourse.bass as bass
import concourse.mybir as mybir
import concourse.tile as tile
from concourse.bass_utils import run_bass_kernel_spmd
from concourse.masks import make_identity

B, S, E = 32, 1024, 512
D, U = 512, 512
NCORES = 8
BL = B // NCORES          # batches per core
P = 128
EC = E // P               # e chunks (4)
UC = U // P               # u chunks (4)
DC = D // P               # d chunks (4)
ST = S // P               # s 128-chunks per batch (8)
NT = BL * ST              # natural [128, 512] tiles per core (32)
HS = 512                  # s per half

F32 = mybir.dt.float32
F32R = mybir.dt.float32r
F16 = mybir.dt.float16
NP16 = np.float16
TANH = mybir.ActivationFunctionType.Tanh
EXP = mybir.ActivationFunctionType.Exp


def build_nc():
    nc = bacc.Bacc("TRN2", target_bir_lowering=False, debug=False,
                   num_devices=NCORES)

    enc = nc.dram_tensor("enc", [BL * S, E], F16, kind="ExternalInput")
    encT = nc.dram_tensor("encT", [E, BL * S], F16, kind="ExternalInput")
    hidT = nc.dram_tensor("hidT", [D, BL], F32R, kind="ExternalInput")
    wa = nc.dram_tensor("wa", [D, U], F16, kind="ExternalInput")
    ua = nc.dram_tensor("ua", [E, U], F16, kind="ExternalInput")
    va = nc.dram_tensor("va", [UC, P], F16, kind="ExternalInput")
    ctx_out = nc.dram_tensor("ctx", [BL, E], F32, kind="ExternalOutput")

    with tile.TileContext(nc) as tc:
        with (
            tc.tile_pool(name="const", bufs=1) as cpool,
            tc.tile_pool(name="nat", bufs=1) as npool,
            tc.tile_pool(name="encT", bufs=1) as tpool,
            tc.tile_pool(name="work", bufs=20) as wpool,
            tc.tile_pool(name="small", bufs=1) as spool,
            tc.tile_pool(name="ps_mm", bufs=4, space="PSUM") as ps_mm,
            tc.tile_pool(name="ps_sc", bufs=2, space="PSUM") as ps_sc,
            tc.tile_pool(name="ps_tr", bufs=2, space="PSUM") as ps_tr,
        ):
            kloop = int(os.environ.get("BASS_ATTN_KLOOP", "1"))
            unroll = int(os.environ.get("BASS_ATTN_UNROLL", "1"))
            import contextlib
            loop_cm = tc.For_i(0, kloop, 1) if kloop > 1 else contextlib.nullcontext()
            with loop_cm:
                for _ in range(unroll):
                    body(nc, tc, cpool, npool, tpool, wpool, spool,
                         ps_mm, ps_sc, ps_tr,
                         enc, encT, hidT, wa, ua, va, ctx_out)

    nc.compile()
    return nc


def body(nc, tc, cpool, npool, tpool, wpool, spool,
         ps_mm, ps_sc, ps_tr, enc, encT, hidT, wa, ua, va, ctx_out):
    # ---- loads, ordered just-in-time for the serial DMA stream ----
    eT_sb = tpool.tile([P, EC, BL * S], F16)       # [e%128, ec, s]
    encT_r = encT.rearrange("(c p) s -> p c s", p=P)

    def load_encT(b, half, ec=None):
        lo = b * S + half * HS
        if ec is None:
            nc.sync.dma_start(out=eT_sb[:, :, lo:lo + HS],
                              in_=encT_r[:, :, lo:lo + HS])
        else:
            nc.sync.dma_start(out=eT_sb[:, ec, lo:lo + HS],
                              in_=encT_r[:, ec, lo:lo + HS])

    nat = npool.tile([P, NT, E], F16)
    enc_r = enc.rearrange("(t p) e -> p t e", p=P)

    def load_nat(b):   # 1 MiB granule = one batch's natural tiles
        nc.sync.dma_start(out=nat[:, ST * b:ST * (b + 1), :],
                          in_=enc_r[:, ST * b:ST * (b + 1), :])

    hidT_sb = cpool.tile([P, DC, BL], F32R)
    nc.sync.dma_start(out=hidT_sb[:], in_=hidT.rearrange("(c p) b -> p c b", p=P))
    wa_sb = cpool.tile([P, DC, U], F16)
    wa_r = wa.rearrange("(c p) u -> p c u", p=P)

    def load_wa(uc):   # 128 KiB granule: the uc-th column block
        nc.sync.dma_start(out=wa_sb[:, :, uc * P:(uc + 1) * P],
                          in_=wa_r[:, :, uc * P:(uc + 1) * P])

    va_sb = cpool.tile([P, UC], F16)
    nc.sync.dma_start(out=va_sb[:], in_=va.rearrange("c p -> p c"))
    load_wa(0)
    load_wa(1)
    load_wa(2)
    load_wa(3)
    ua_sb = cpool.tile([P, EC, U], F16)
    ua_r = ua.rearrange("(c p) u -> p c u", p=P)

    def load_ua(ec):
        nc.sync.dma_start(out=ua_sb[:, ec, :], in_=ua_r[:, ec, :])

    # first batch-half arrives in ec-granules interleaved with ua so the
    # very first matmul can start after ~256 KiB instead of ~1 MiB
    load_ua(0)
    load_encT(0, 0, ec=0)
    load_ua(1)
    load_encT(0, 0, ec=1)
    load_ua(2)
    load_encT(0, 0, ec=2)
    load_ua(3)
    load_encT(0, 0, ec=3)
    load_encT(1, 0)
    load_encT(2, 0)
    load_encT(3, 0)
    for b in range(BL):
        load_encT(b, 1)
    for b in range(BL):
        load_nat(b)

    # ---- small constants / setup (gpsimd + vector, off critical path) ----
    ident32 = cpool.tile([P, P], F32)
    make_identity(nc, ident32[:])
    ident = cpool.tile([P, P], F16)
    nc.vector.tensor_copy(ident[:], ident32[:])
    hid16 = cpool.tile([P, DC, BL], F16)
    nc.vector.tensor_copy(hid16[:], hidT_sb[:])
    decT_sb = cpool.tile([P, UC, BL], F32)

    def dec_proj():
        # dec_projT[u, b] = sum_d Wa[d, u] * hid[b, d]; fills the initial
        # DMA-wait bubble (wa uc-chunks land first).
        for uc in range(UC):
            ps = ps_sc.tile([P, BL], F32, tag="sc")
            for dc in range(DC):
                nc.tensor.matmul(
                    ps[:], wa_sb[:, dc, uc * P:(uc + 1) * P],
                    hid16[:, dc, :],
                    start=(dc == 0), stop=(dc == DC - 1),
                )
            nc.vector.tensor_copy(decT_sb[:, uc, :], ps[:])

    alpha = spool.tile([P, S], F16, tag="alpha")
    esums = [spool.tile([P, 1], F32, tag=f"es{h}", name=f"esum{h}")
             for h in range(2)]
    aT = spool.tile([P, NT], F16, tag="aT")      # col 4t+b = batch b, chunk t

    e_tiles = {}

    def enc_proj(b, half):
        sl = slice(b * S + half * HS, b * S + (half + 1) * HS)
        for uc in range(UC):
            psm = ps_mm.tile([P, HS], F32, tag="mm")
            for ec in range(EC):
                nc.tensor.matmul(
                    psm[:],
                    ua_sb[:, ec, uc * P:(uc + 1) * P],
                    eT_sb[:, ec, sl],
                    start=(ec == 0), stop=(ec == EC - 1),
                )
            e_t = wpool.tile([P, HS], F16, tag="e")
            nc.scalar.activation(e_t[:], psm[:], TANH,
                                 bias=decT_sb[:, uc, b:b + 1])
            e_tiles[(b, half, uc)] = e_t

    def scores(half):
        # per-batch M=1 matmuls; exp lands in the shared alpha tile at
        # partition row 32b (ACT handles the base-partition shift).
        osl = slice(half * HS, (half + 1) * HS)
        for b in range(BL):
            pss = ps_sc.tile([1, HS], F32, tag="sc")
            for uc in range(UC):
                nc.tensor.matmul(
                    pss[:], va_sb[:, uc:uc + 1],
                    e_tiles[(b, half, uc)][:],
                    start=(uc == 0), stop=(uc == UC - 1),
                )
            r = slice(32 * b, 32 * b + 1)
            nc.scalar.activation(alpha[r, osl], pss[:], EXP,
                                 accum_out=esums[half][r, :])

    def transp(half):
        # alphaT columns for the context lhsT: one [128,128] PE transpose
        # per s-chunk covers all 4 batches (their rows 32b become cols 32b).
        for t in range(half * 4, half * 4 + 4):
            psa = ps_tr.tile([P, P], F16, tag="tr")
            nc.tensor.transpose(psa[:], alpha[:, t * P:(t + 1) * P], ident[:])
            nc.vector.tensor_copy(aT[:, 4 * t:4 * t + 4], psa[:, 0:P:32])

    ctxh = [spool.tile([P, E], F32, tag=f"cxh{h}", name=f"ctxh{h}")
            for h in range(2)]

    def context(half):
        for b in range(BL):
            psc = ps_sc.tile([1, E], F32, tag="sc")
            for t in range(half * 4, half * 4 + 4):
                nc.tensor.matmul(
                    psc[:],
                    aT[:, 4 * t + b:4 * t + b + 1],
                    nat[:, b * ST + t, :],
                    start=(t == half * 4), stop=(t == half * 4 + 3),
                )
            r = slice(32 * b, 32 * b + 1)
            nc.vector.tensor_copy(ctxh[half][r, :], psc[:])

    # ---- issue order = PE execution order ----
    dec_proj()                 # runs while ua/encT stream in
    for b in range(BL):
        enc_proj(b, 0)
    scores(0)
    for b in range(BL):
        enc_proj(b, 1)
    transp(0)
    context(0)
    scores(1)
    transp(1)
    context(1)

    # softmax normalizer folded into the context evacuation
    ssum = spool.tile([P, 1], F32, tag="ssum")
    nc.vector.tensor_add(ssum[:], esums[0][:], esums[1][:])
    rsum = spool.tile([P, 1], F32, tag="rsum")
    nc.vector.reciprocal(rsum[:], ssum[:])
    ctx_sb = spool.tile([P, E], F32, tag="ctx")
    nc.vector.tensor_add(ctx_sb[:], ctxh[0][:], ctxh[1][:])
    nc.vector.tensor_scalar_mul(ctx_sb[:], ctx_sb[:], rsum[:])
    nc.sync.dma_start(out=ctx_out[:, :], in_=ctx_sb[0:P:32, :])


_NC_CACHE = None


def _in_maps(outp_enc, hidden_dec, Wa, Ua, Va):
    outp_enc = np.ascontiguousarray(outp_enc, dtype=np.float32)
    hidden_dec = np.ascontiguousarray(hidden_dec, dtype=np.float32)
    wa = np.ascontiguousarray(Wa, dtype=NP16)
    ua = np.ascontiguousarray(Ua, dtype=NP16)
    va = np.ascontiguousarray(Va, dtype=NP16).reshape(UC, P)

    in_maps = []
    for c in range(NCORES):
        bs = slice(c * BL, (c + 1) * BL)
        enc_c = outp_enc[bs].reshape(BL * S, E)
        in_maps.append({
            "enc": enc_c.astype(NP16),
            "encT": np.ascontiguousarray(enc_c.T).astype(NP16),
            "hidT": np.ascontiguousarray(hidden_dec[bs].T),
            "wa": wa, "ua": ua, "va": va,
        })
    return in_maps


def run_spmd(outp_enc, hidden_dec, Wa, Ua, Va, **kwargs):
    global _NC_CACHE
    if _NC_CACHE is None:
        _NC_CACHE = build_nc()
    res = run_bass_kernel_spmd(
        _NC_CACHE, _in_maps(outp_enc, hidden_dec, Wa, Ua, Va),
        core_ids=list(range(NCORES)), **kwargs,
    )
    out = np.concatenate([res.results[c]["ctx"] for c in range(NCORES)], axis=0)
    return out.astype(np.float32), res


def kernel(outp_enc, hidden_dec, Wa, Ua, Va):
    out, _ = run_spmd(outp_enc, hidden_dec, Wa, Ua, Va)
    return out


if __name__ == "__main__":
    rng = np.random.default_rng(0)
    inputs = {
        "outp_enc": rng.standard_normal((B, S, E), dtype=np.float32),
        "hidden_dec": rng.standard_normal((B, D), dtype=np.float32),
        "Wa": (rng.standard_normal((D, U), dtype=np.float32) / np.sqrt(D)),
        "Ua": (rng.standard_normal((E, U), dtype=np.float32) / np.sqrt(E)),
        "Va": (rng.standard_normal((U,), dtype=np.float32) / np.sqrt(U)),
    }
    out = kernel(**inputs)
    print("out", out.shape, out.dtype)


# revision 35
# speedup vs baseline: 1.3180x; 1.3180x over previous
"""Bahdanau additive attention on 8 Trainium2 NeuronCores.

reference:
    dec_proj = hidden_dec @ Wa                       # [B, U]
    enc_proj = einsum("bse,eu->bsu", outp_enc, Ua)   # [B, S, U]
    e        = tanh(enc_proj + dec_proj[:, None, :])
    scores   = einsum("bsu,u->bs", e, Va)
    alpha    = softmax(scores, axis=-1)
    context  = einsum("bs,bse->be", alpha, outp_enc)  # [B, E]

Sharding: data-parallel on batch. B=32 over 8 cores -> 4 batches/core.
Weights (Wa, Ua, Va) replicated; no collectives.

Per-core plan (4 local batches, S=1024, E=U=D=512), processed
half-by-half (512 s at a time) across all 4 batches:
  - host ships TWO fp16 layouts of the enc shard: transposed [e,s] for
    the enc_proj matmuls and natural [s,e] for the context matmuls
    (halves HBM traffic vs f32 natural).
  - enc_proj in layout [u, s]: lhsT = Ua chunk, rhs = encT; tanh's
    dec_proj bias is a per-partition scalar on the scalar engine.
  - scores = Va . tanh(...) via per-batch M=1 PE matmuls; exp lands in
    ONE shared alpha tile [128, S] at partition row 32b per batch (the
    scalar engine handles the base-partition shift), with accum_out
    collecting the softmax normalizer.
  - 8 shared PE transposes (one per 128-s-chunk, covering all 4
    batches at once) yield the alphaT columns for the context lhsT.
  - context accumulates per (batch, half) in PSUM, is evacuated to
    SBUF rows 32b, halves summed once; 1/sum(exp) is folded into the
    final scale and the output ships as a single partition-strided DMA.
  - alpha stays unnormalized. No max-subtraction: |scores| <=
    ||Va||_1 with tanh in (-1,1), far from fp32 exp overflow.
  - loads stream just-in-time (wa in uc-granules so dec_proj fills the
    initial DMA bubble; first encT batch-half in ec-granules); in the
    repeat loop, next-iteration loads overlap this iteration's tail.
fp16 on the scores path adds ~2-4e-4 relative error; fp16 natural
layout adds ~1e-4 on the context sum.
"""

import os

import numpy as np

import concourse.bacc as bacc
import concourse.bass as bass
import concourse.mybir as mybir
import concourse.tile as tile
from concourse.bass_utils import run_bass_kernel_spmd
from concourse.masks import make_identity

B, S, E = 32, 1024, 512
D, U = 512, 512
NCORES = 8
BL = B // NCORES          # batches per core
P = 128
EC = E // P               # e chunks (4)
UC = U // P               # u chunks (4)
DC = D // P               # d chunks (4)
ST = S // P               # s 128-chunks per batch (8)
NT = BL * ST              # natural [128, 512] tiles per core (32)
HS = 512                  # s per half

F32 = mybir.dt.float32
F32R = mybir.dt.float32r
F16 = mybir.dt.float16
NP16 = np.float16
TANH = mybir.ActivationFunctionType.Tanh
EXP = mybir.ActivationFunctionType.Exp


def build_nc():
    nc = bacc.Bacc("TRN2", target_bir_lowering=False, debug=False,
                   num_devices=NCORES)

    enc = nc.dram_tensor("enc", [BL * S, E], F16, kind="ExternalInput")
    encT = nc.dram_tensor("encT", [E, BL * S], F16, kind="ExternalInput")
    hidT = nc.dram_tensor("hidT", [D, BL], F32R, kind="ExternalInput")
    wa = nc.dram_tensor("wa", [D, U], F16, kind="ExternalInput")
    ua = nc.dram_tensor("ua", [E, U], F16, kind="ExternalInput")
    va = nc.dram_tensor("va", [UC, P], F16, kind="ExternalInput")
    ctx_out = nc.dram_tensor("ctx", [BL, E], F32, kind="ExternalOutput")

    with tile.TileContext(nc) as tc:
        with (
            tc.tile_pool(name="const", bufs=1) as cpool,
            tc.tile_pool(name="nat", bufs=1) as npool,
            tc.tile_pool(name="encT", bufs=1) as tpool,
            tc.tile_pool(name="work", bufs=20) as wpool,
            tc.tile_pool(name="small", bufs=1) as spool,
            tc.tile_pool(name="ps_mm", bufs=3, space="PSUM") as ps_mm,
            tc.tile_pool(name="ps_sc", bufs=2, space="PSUM") as ps_sc,
            tc.tile_pool(name="ps_tr", bufs=2, space="PSUM") as ps_tr,
        ):
            kloop = int(os.environ.get("BASS_ATTN_KLOOP", "1"))
            import contextlib
            loop_cm = tc.For_i(0, kloop, 1) if kloop > 1 else contextlib.nullcontext()
            with loop_cm:
                body(nc, tc, cpool, npool, tpool, wpool, spool,
                     ps_mm, ps_sc, ps_tr,
                     enc, encT, hidT, wa, ua, va, ctx_out)

    nc.compile()
    return nc


def body(nc, tc, cpool, npool, tpool, wpool, spool,
         ps_mm, ps_sc, ps_tr, enc, encT, hidT, wa, ua, va, ctx_out):
    # ---- loads, ordered just-in-time for the serial DMA stream ----
    eT_sb = tpool.tile([P, EC, BL * S], F16)       # [e%128, ec, s]
    encT_r = encT.rearrange("(c p) s -> p c s", p=P)

    def load_encT(b, half, ec=None):
        lo = b * S + half * HS
        if ec is None:
            nc.sync.dma_start(out=eT_sb[:, :, lo:lo + HS],
                              in_=encT_r[:, :, lo:lo + HS])
        else:
            nc.sync.dma_start(out=eT_sb[:, ec, lo:lo + HS],
                              in_=encT_r[:, ec, lo:lo + HS])

    nat = npool.tile([P, NT, E], F16)
    enc_r = enc.rearrange("(t p) e -> p t e", p=P)

    def load_nat(b):   # 1 MiB granule = one batch's natural tiles
        nc.sync.dma_start(out=nat[:, ST * b:ST * (b + 1), :],
                          in_=enc_r[:, ST * b:ST * (b + 1), :])

    hidT_sb = cpool.tile([P, DC, BL], F32R)
    nc.sync.dma_start(out=hidT_sb[:], in_=hidT.rearrange("(c p) b -> p c b", p=P))
    wa_sb = cpool.tile([P, DC, U], F16)
    wa_r = wa.rearrange("(c p) u -> p c u", p=P)

    def load_wa(uc):   # 128 KiB granule: the uc-th column block
        nc.sync.dma_start(out=wa_sb[:, :, uc * P:(uc + 1) * P],
                          in_=wa_r[:, :, uc * P:(uc + 1) * P])

    va_sb = cpool.tile([P, UC], F16)
    nc.sync.dma_start(out=va_sb[:], in_=va.rearrange("c p -> p c"))
    load_wa(0)
    load_wa(1)
    load_wa(2)
    load_wa(3)
    ua_sb = cpool.tile([P, EC, U], F16)
    ua_r = ua.rearrange("(c p) u -> p c u", p=P)

    def load_ua(ec):
        nc.sync.dma_start(out=ua_sb[:, ec, :], in_=ua_r[:, ec, :])

    # first batch-half arrives in ec-granules interleaved with ua so the
    # very first matmul can start after ~256 KiB instead of ~1 MiB
    load_ua(0)
    load_encT(0, 0, ec=0)
    load_ua(1)
    load_encT(0, 0, ec=1)
    load_ua(2)
    load_encT(0, 0, ec=2)
    load_ua(3)
    load_encT(0, 0, ec=3)
    load_encT(1, 0)
    load_encT(2, 0)
    load_encT(3, 0)
    for b in range(BL):
        load_encT(b, 1)
    for b in range(BL):
        load_nat(b)

    # ---- small constants / setup (gpsimd + vector, off critical path) ----
    ident32 = cpool.tile([P, P], F32)
    make_identity(nc, ident32[:])
    ident = cpool.tile([P, P], F16)
    nc.vector.tensor_copy(ident[:], ident32[:])
    hid16 = cpool.tile([P, DC, BL], F16)
    nc.vector.tensor_copy(hid16[:], hidT_sb[:])
    decT_sb = cpool.tile([P, UC, BL], F32)

    def dec_proj():
        # dec_projT[u, b] = sum_d Wa[d, u] * hid[b, d]; fills the initial
        # DMA-wait bubble (wa uc-chunks land first).
        for uc in range(UC):
            ps = ps_sc.tile([P, BL], F32, tag="sc")
            for dc in range(DC):
                nc.tensor.matmul(
                    ps[:], wa_sb[:, dc, uc * P:(uc + 1) * P],
                    hid16[:, dc, :],
                    start=(dc == 0), stop=(dc == DC - 1),
                )
            nc.vector.tensor_copy(decT_sb[:, uc, :], ps[:])

    alpha = spool.tile([P, S], F16, tag="alpha")
    esums = [spool.tile([P, 1], F32, tag=f"es{h}", name=f"esum{h}")
             for h in range(2)]
    aT = spool.tile([P, NT], F16, tag="aT")      # col 4t+b = batch b, chunk t

    e_tiles = {}

    def enc_proj(b, half):
        sl = slice(b * S + half * HS, b * S + (half + 1) * HS)
        for uc in range(UC):
            psm = ps_mm.tile([P, HS], F32, tag="mm")
            for ec in range(EC):
                nc.tensor.matmul(
                    psm[:],
                    ua_sb[:, ec, uc * P:(uc + 1) * P],
                    eT_sb[:, ec, sl],
                    start=(ec == 0), stop=(ec == EC - 1),
                )
            e_t = wpool.tile([P, HS], F16, tag="e")
            nc.scalar.activation(e_t[:], psm[:], TANH,
                                 bias=decT_sb[:, uc, b:b + 1])
            e_tiles[(b, half, uc)] = e_t

    def scores(half):
        # per-batch M=1 matmuls; exp lands in the shared alpha tile at
        # partition row 32b (ACT handles the base-partition shift).
        osl = slice(half * HS, (half + 1) * HS)
        for b in range(BL):
            pss = ps_sc.tile([1, HS], F32, tag="sc")
            for uc in range(UC):
                nc.tensor.matmul(
                    pss[:], va_sb[:, uc:uc + 1],
                    e_tiles[(b, half, uc)][:],
                    start=(uc == 0), stop=(uc == UC - 1),
                )
            r = slice(32 * b, 32 * b + 1)
            nc.scalar.activation(alpha[r, osl], pss[:], EXP,
                                 accum_out=esums[half][r, :])

    def transp(half):
        # alphaT columns for the context lhsT: one [128,128] PE transpose
        # per s-chunk covers all 4 batches (their rows 32b become cols 32b).
        for t in range(half * 4, half * 4 + 4):
            psa = ps_tr.tile([P, P], F16, tag="tr")
            nc.tensor.transpose(psa[:], alpha[:, t * P:(t + 1) * P], ident[:])
            nc.vector.tensor_copy(aT[:, 4 * t:4 * t + 4], psa[:, 0:P:32])

    ctxh = [spool.tile([P, E], F32, tag=f"cxh{h}", name=f"ctxh{h}")
            for h in range(2)]

    def context(half):
        for b in range(BL):
            psc = ps_sc.tile([1, E], F32, tag="sc")
            for t in range(half * 4, half * 4 + 4):
                nc.tensor.matmul(
                    psc[:],
                    aT[:, 4 * t + b:4 * t + b + 1],
                    nat[:, b * ST + t, :],
                    start=(t == half * 4), stop=(t == half * 4 + 3),
                )
            r = slice(32 * b, 32 * b + 1)
            nc.scalar.copy(ctxh[half][r, :], psc[:])

    # ---- issue order = PE execution order ----
    dec_proj()                 # runs while ua/encT stream in
    for b in range(BL):
        enc_proj(b, 0)
    scores(0)
    for b in range(BL):
        enc_proj(b, 1)
    transp(0)
    context(0)
    scores(1)
    transp(1)
    context(1)

    # softmax normalizer folded into the context evacuation
    ssum = spool.tile([P, 1], F32, tag="ssum")
    nc.vector.tensor_add(ssum[:], esums[0][:], esums[1][:])
    rsum = spool.tile([P, 1], F32, tag="rsum")
    nc.vector.reciprocal(rsum[:], ssum[:])
    ctx_sb = spool.tile([P, E], F32, tag="ctx")
    nc.vector.tensor_add(ctx_sb[:], ctxh[0][:], ctxh[1][:])
    nc.vector.tensor_scalar_mul(ctx_sb[:], ctx_sb[:], rsum[:])
    nc.sync.dma_start(out=ctx_out[:, :], in_=ctx_sb[0:P:32, :])


_NC_CACHE = None


def _in_maps(outp_enc, hidden_dec, Wa, Ua, Va):
    outp_enc = np.ascontiguousarray(outp_enc, dtype=np.float32)
    hidden_dec = np.ascontiguousarray(hidden_dec, dtype=np.float32)
    wa = np.ascontiguousarray(Wa, dtype=NP16)
    ua = np.ascontiguousarray(Ua, dtype=NP16)
    va = np.ascontiguousarray(Va, dtype=NP16).reshape(UC, P)

    in_maps = []
    for c in range(NCORES):
        bs = slice(c * BL, (c + 1) * BL)
        enc_c = outp_enc[bs].reshape(BL * S, E)
        in_maps.append({
            "enc": enc_c.astype(NP16),
            "encT": np.ascontiguousarray(enc_c.T).astype(NP16),
            "hidT": np.ascontiguousarray(hidden_dec[bs].T),
            "wa": wa, "ua": ua, "va": va,
        })
    return in_maps


def run_spmd(outp_enc, hidden_dec, Wa, Ua, Va, **kwargs):
    global _NC_CACHE
    if _NC_CACHE is None:
        _NC_CACHE = build_nc()
    res = run_bass_kernel_spmd(
        _NC_CACHE, _in_maps(outp_enc, hidden_dec, Wa, Ua, Va),
        core_ids=list(range(NCORES)), **kwargs,
    )
    out = np.concatenate([res.results[c]["ctx"] for c in range(NCORES)], axis=0)
    return out.astype(np.float32), res


def kernel(outp_enc, hidden_dec, Wa, Ua, Va):
    out, _ = run_spmd(outp_enc, hidden_dec, Wa, Ua, Va)
    return out


if __name__ == "__main__":
    rng = np.random.default_rng(0)
    inputs = {
        "outp_enc": rng.standard_normal((B, S, E), dtype=np.float32),
        "hidden_dec": rng.standard_normal((B, D), dtype=np.float32),
        "Wa": (rng.standard_normal((D, U), dtype=np.float32) / np.sqrt(D)),
        "Ua": (rng.standard_normal((E, U), dtype=np.float32) / np.sqrt(E)),
        "Va": (rng.standard_normal((U,), dtype=np.float32) / np.sqrt(U)),
    }
    out = kernel(**inputs)
    print("out", out.shape, out.dtype)
